# revision 41
# baseline (speedup 1.0000x reference)
"""Causal self-attention Trainium2 kernel (B=4, N=2048, D=1024, H=16, HD=64).

Sharding: tensor-parallel over heads — 8 cores x 2 heads each, all 4 batches.
Each core computes q/k/v projections for its 2 heads, causal-softmax
attention, and its partial contribution to the output projection
(sa_local @ Wout[:, cols].T). Host sums the 8 partials and adds bout.

Everything on-chip is kept "transposed" ([feature, token]) so no on-device
transposes are needed:
  - scores^T[k, q] = matmul(lhsT=kT_block, rhs=qT_chunk)
  - softmax denominator comes free as row 64 of the PV matmul by augmenting
    v with a ones column
  - U^T = v_aug^T @ expS^T accumulates over k-tiles in PSUM
  - out^T[j, n] = matmul(lhsT=WoutT_cols, rhs=saT)
k/q/v biases are folded into the projection matmuls via a ones row of x.
k, q, es, v, sa and the output-projection operands are bf16; the softmax
exp is skipped-max (scores ~N(0,1), softmax is shift-invariant).
"""

import os
import sys

for _p in ("/opt/trn_rl_repo", "/root/.axon_site/_ro/trn_rl_repo"):
    if os.path.isdir(_p) and _p not in sys.path:
        sys.path.insert(0, _p)
        break

import ml_dtypes
import numpy as np

import concourse.bacc as bacc
import concourse.tile as tile
from concourse import mybir
from concourse.bass_utils import run_bass_kernel_spmd

B, N, D, H = 4, 2048, 1024, 16
HD = D // H  # 64
NCORES = 8
HLOC = H // NCORES  # 2 local heads per core
BN = B * N  # 8192
QC = 512  # q-chunk width (PSUM bank)
KT = 128  # k-tile height
NQC = N // QC  # 4 q-chunks per batch
NKT = N // KT  # 16 k-tiles per batch

F32 = mybir.dt.float32
F32R = mybir.dt.float32r
BF16 = mybir.dt.bfloat16
BF16NP = ml_dtypes.bfloat16

LAST_RUN = None  # BassKernelResults of the most recent run (for test harness)


def _build_program():
    nc = bacc.Bacc("TRN2", num_devices=NCORES)

    # Per-core inputs (same shapes on every core, different values).
    xt = nc.dram_tensor("xt", [HLOC, HD + 1, BN], F32R, kind="ExternalInput")
    xtb = nc.dram_tensor("xtb", [HLOC, HD + 1, BN], BF16, kind="ExternalInput")
    wk = nc.dram_tensor("wk", [HD + 1, HLOC, HD], F32R, kind="ExternalInput")
    wq = nc.dram_tensor("wq", [HD + 1, HLOC, HD], F32R, kind="ExternalInput")
    wv = nc.dram_tensor("wv", [HD + 1, HLOC, HD + 2], BF16, kind="ExternalInput")
    wo = nc.dram_tensor("wo", [HLOC * HD, D], BF16, kind="ExternalInput")
    # -40 strictly above the causal diagonal of a 128x128 block (c < r),
    # 0 elsewhere; added to diagonal score blocks pre-exp via I128.T @ msk.
    one64 = nc.dram_tensor("one64", [1, HD], BF16, kind="ExternalInput")
    msk = nc.dram_tensor("msk", [KT, KT], BF16, kind="ExternalInput")
    i128 = nc.dram_tensor("i128", [KT, KT], BF16, kind="ExternalInput")
    # yt layout: [partition, out-block jc, token] so one DMA can cover
    # multiple jc blocks with a partition-major access pattern.
    yt = nc.dram_tensor("yt", [128, D // 128, BN], BF16, kind="ExternalOutput")

    with tile.TileContext(nc) as tc:
        with (
            nc.allow_low_precision(reason="bf16/f32r attention pipeline"),
            tc.tile_pool(name="const", bufs=1) as const,
            tc.tile_pool(name="kq", bufs=2) as kq_pool,
            tc.tile_pool(name="vp", bufs=2) as v_pool,
            tc.tile_pool(name="xp", bufs=2) as x_pool,
            tc.tile_pool(name="es", bufs=9) as es_pool,
            tc.tile_pool(name="sa", bufs=2) as sa_pool,
            tc.tile_pool(name="rq", bufs=3) as rq_pool,
            tc.tile_pool(name="yout", bufs=4) as y_pool,
            tc.tile_pool(name="pbig", bufs=2, space="PSUM") as big_pool,
            tc.tile_pool(name="pmed", bufs=2, space="PSUM") as med_pool,
            tc.tile_pool(name="psu", bufs=2, space="PSUM") as psu_pool,
        ):
            # --- resident weight tiles (only wk/wq block the first step's
            # projections; the rest load via the gpsimd DGE queue) ---
            wk_sb = const.tile([HD + 1, HLOC, HD], F32R, tag="wk")
            nc.sync.dma_start(out=wk_sb, in_=wk.ap())
            wq_sb = const.tile([HD + 1, HLOC, HD], F32R, tag="wq")
            nc.scalar.dma_start(out=wq_sb, in_=wq.ap())
            one64_sb = const.tile([1, HD], BF16, tag="one64")
            nc.gpsimd.dma_start(out=one64_sb, in_=one64.ap())
            msk_sb = const.tile([KT, KT], BF16, tag="msk")
            nc.gpsimd.dma_start(out=msk_sb, in_=msk.ap())
            i128_sb = const.tile([KT, KT], BF16, tag="i128")
            nc.gpsimd.dma_start(out=i128_sb, in_=i128.ap())
            wv_sb = const.tile([HD + 1, HLOC, HD + 2], BF16, tag="wv")
            nc.gpsimd.dma_start(out=wv_sb, in_=wv.ap())
            wo_sb = const.tile([HLOC * HD, D], BF16, tag="wo")
            nc.gpsimd.dma_start(out=wo_sb, in_=wo.ap())

            # Per-batch SBUF state (k/q hold both heads stacked on 128
            # partitions), created when proj units are emitted.
            stb = {}

            def proj_units(b):
                """k/q/v projections for batch b (both heads) as a list of
                closures (one PSUM slot each) to interleave with the
                previous batch's attention."""
                boff = b * N
                state = {"xl": {}, "xlb": {}, "v": {}}
                stb[b] = state

                def mk(l):
                    def run():
                        xl = x_pool.tile([HD + 1, N], F32R, tag=f"xt{l}")
                        h = N // 2
                        nc.sync.dma_start(
                            out=xl[:, 0:h], in_=xt.ap()[l][:, boff : boff + h]
                        )
                        nc.sync.dma_start(
                            out=xl[:, h:N],
                            in_=xt.ap()[l][:, boff + h : boff + N],
                        )
                        xlb = x_pool.tile([HD + 1, N], BF16, tag=f"xtb{l}")
                        nc.scalar.dma_start(
                            out=xlb, in_=xtb.ap()[l][:, boff : boff + N]
                        )
                        state["xl"][l] = xl
                        state["xlb"][l] = xlb
                        klt = kq_pool.tile([HD, N], BF16, tag=f"k{l}")
                        qlt = kq_pool.tile([HD, N], BF16, tag=f"q{l}")
                        state[f"k{l}"] = klt
                        state[f"q{l}"] = qlt
                        vlt = v_pool.tile([KT, NKT, HD + 1], BF16, tag=f"v{l}")
                        state["v"][l] = vlt
                    return run

                def kq_unit(jp, which, l):
                    def run():
                        dst = state[f"{which}{l}"]
                        w_sb = wk_sb if which == "k" else wq_sb
                        psk = big_pool.tile([HD, 2 * QC], F32, tag="big")
                        for half in range(2):
                            j = 2 * jp + half
                            sl = slice(j * QC, (j + 1) * QC)
                            osl = slice(half * QC, (half + 1) * QC)
                            nc.tensor.matmul(
                                psk[:, osl],
                                w_sb[:, l, :],
                                state["xl"][l][:, sl],
                                start=True, stop=True,
                            )
                        ksl = slice(2 * jp * QC, 2 * (jp + 1) * QC)
                        nc.vector.tensor_copy(out=dst[:, ksl], in_=psk)
                    return run

                def v_unit(l, g):
                    def run():
                        psv = med_pool.tile([KT, 4, KT], F32, tag="med")
                        for gg in range(4):
                            kj = 4 * g + gg
                            nc.tensor.matmul(
                                psv[:, gg, 0 : HD + 2],
                                state["xlb"][l][:, kj * KT : (kj + 1) * KT],
                                wv_sb[:, l, :],
                                start=True, stop=True,
                            )
                        nc.vector.tensor_copy(
                            out=state["v"][l][:, 4 * g : 4 * (g + 1), :],
                            in_=psv[:, :, 0 : HD + 1],
                        )
                    return run

                units = [mk(0), mk(1)]
                units += [
                    kq_unit(jp, w, l)
                    for l in range(HLOC)
                    for jp in range(NQC // 2)
                    for w in ("k", "q")
                ]
                units += [v_unit(l, g) for l in range(HLOC) for g in range(NKT // 4)]
                return units

            # diag block (j, t) -> slot index; slots 0-7 live in a big PSUM
            # tile, slots 8-9 in a med tile. t == j is the triangular block.
            TRI = (0, 1, 3, 6)

            def attn_emit(i, background, on_qc=None):
                """Attention for step i; pops background units between
                score/PV pairs."""
                b, l = divmod(i, HLOC)
                state = stb[b]
                k_sb = state[f"k{l}"]
                q_sb = state[f"q{l}"]
                v_sb = state["v"][l]

                def emit_scores(unit):
                    kind, qc, t2 = unit
                    qsl = slice(qc * QC, (qc + 1) * QC)
                    if kind == "full":
                        pss = big_pool.tile([KT, 2 * QC], F32, tag="big")
                        es = es_pool.tile([KT, 2 * QC], BF16, tag="es")
                        for half in range(2):
                            kj = 2 * t2 + half
                            nc.tensor.matmul(
                                pss[:, half * QC : (half + 1) * QC],
                                k_sb[:, kj * KT : (kj + 1) * KT],
                                q_sb[:, qsl],
                                start=True, stop=True,
                            )
                        nc.scalar.activation(
                            out=es, in_=pss, func=mybir.ActivationFunctionType.Exp
                        )
                        return es
                    # fine diagonal unit: 10 [128,128] blocks (j, t<=j) for
                    # q columns j of this q-chunk; triangular blocks get a
                    # -40 additive mask via a matmul before exp.
                    pssb = big_pool.tile([KT, 8, KT], F32, tag="big")
                    pssm = med_pool.tile([KT, 4, KT], F32, tag="med")
                    esb = es_pool.tile([KT, 8, KT], BF16, tag="es")
                    esm = es_pool.tile([KT, 2, KT], BF16, tag="esd")
                    for j in range(4):
                        q0 = qc * QC + j * KT
                        for t in range(j + 1):
                            s = TRI[j] + t
                            dst = pssb[:, s, :] if s < 8 else pssm[:, s - 8, :]
                            kj = 4 * qc + t
                            nc.tensor.matmul(
                                dst,
                                k_sb[:, kj * KT : (kj + 1) * KT],
                                q_sb[:, q0 : q0 + KT],
                                start=True, stop=(t != j),
                            )
                            if t == j:
                                nc.tensor.matmul(
                                    dst, i128_sb, msk_sb,
                                    start=False, stop=True,
                                )
                    nc.scalar.activation(
                        out=esb, in_=pssb,
                        func=mybir.ActivationFunctionType.Exp,
                    )
                    nc.scalar.activation(
                        out=esm, in_=pssm[:, 0:2, :],
                        func=mybir.ActivationFunctionType.Exp,
                    )
                    return (esb, esm)

                psu_map = {}

                def emit_pv(unit, es):
                    kind, qc, t2 = unit
                    qsl = slice(qc * QC, (qc + 1) * QC)
                    if kind == "full":
                        for half in range(2):
                            kj = 2 * t2 + half
                            nc.tensor.matmul(
                                psu_map[qc],
                                v_sb[:, kj, :],
                                es[:, half * QC : (half + 1) * QC],
                                start=(kj == 0),
                                stop=False,
                            )
                        return
                    esb, esm = es
                    for j in range(4):
                        for t in range(j + 1):
                            s = TRI[j] + t
                            src = esb[:, s, :] if s < 8 else esm[:, s - 8, :]
                            kj = 4 * qc + t
                            nc.tensor.matmul(
                                psu_map[qc][:, j * KT : (j + 1) * KT],
                                v_sb[:, kj, :],
                                src,
                                start=(kj == 0),
                                # for qc 0 each 128-col region is its own
                                # accumulation group (no preceding fulls) —
                                # close it at the column's last block
                                stop=(j == 3 and t == 3)
                                or (qc == 0 and t == j),
                            )
                    if True:
                        # softmax denominator is row 64 of psu; normalize the
                        # finished q-chunk straight out of PSUM into saT.
                        # engine ops misbehave at partition offsets on this
                        # backend, so: evacuate psu to SBUF at base 0, DMA
                        # the denominator row down to partition 0, invert it
                        # there, and broadcast via a ones-column matmul.
                        psu = psu_map[qc]
                        saT = sa_map[b]
                        u65 = rq_pool.tile([HD + 1, QC], F32, tag="u65")
                        nc.vector.tensor_copy(out=u65, in_=psu)
                        dn = rq_pool.tile([1, QC], F32, tag="dn")
                        nc.gpsimd.dma_start(out=dn, in_=u65[HD : HD + 1, :])
                        rq = rq_pool.tile([1, QC], F32, tag="rq")
                        nc.vector.reciprocal_approx_fast(out=rq, in_=dn)
                        # f32r matmul operands must be produced as f32r;
                        # round 1/den to bf16 on the idle gpsimd instead
                        rqb = rq_pool.tile([1, QC], BF16, tag="rqb")
                        nc.gpsimd.tensor_copy(out=rqb, in_=rq)
                        psb = med_pool.tile([HD, QC], F32, tag="med")
                        nc.tensor.matmul(
                            psb, one64_sb, rqb, start=True, stop=True,
                        )
                        if l == 0:
                            nc.vector.tensor_mul(
                                out=saT[0:HD, qsl], in0=u65[0:HD, :], in1=psb
                            )
                        else:
                            # DVE lanes cannot shift partitions: stage head 1
                            # in SBUF and DMA-shift into rows 64-127 of saT.
                            sa_tmp = rq_pool.tile([HD, QC], BF16, tag="sat")
                            nc.vector.tensor_mul(
                                out=sa_tmp, in0=u65[0:HD, :], in1=psb
                            )
                            nc.gpsimd.dma_start(
                                out=saT[HD : 2 * HD, qsl], in_=sa_tmp
                            )
                        if on_qc is not None:
                            on_qc(qc)

                work = []
                for qc in range(NQC):
                    work += [("full", qc, t2) for t2 in range(2 * qc)]
                    work += [("diag", qc, 0)]
                npairs = len(work)
                pending = []
                for idx, unit in enumerate(work):
                    qc = unit[1]
                    if qc not in psu_map:
                        psu_t = psu_pool.tile([HD + 1, QC], F32, tag="psu")
                        psu_map[qc] = psu_t
                    if len(pending) >= 7:
                        u0, e0 = pending.pop(0)
                        emit_pv(u0, e0)
                    es = emit_scores(unit)
                    pending.append((unit, es))
                    # interleave background work (other steps' proj/outproj)
                    remaining = npairs - idx
                    take = -(-len(background) // remaining)  # ceil
                    for _ in range(take):
                        if background:
                            background.pop(0)()
                for u0, e0 in pending:
                    emit_pv(u0, e0)
                while background:
                    background.pop(0)()

            NJC2 = D // 256  # 4 output jc-pairs

            def outproj_units(b):
                """Output projection for batch b: 16 units keyed (qc, jc2),
                released per q-chunk. Each jc2 owns a [128, 2, 2048] bf16
                tile, DMA'd after its qc=3 unit."""
                boff = b * N
                y_map = {}

                def y_unit(qc, jc2):
                    def run():
                        saT = sa_map[b]
                        if jc2 not in y_map:
                            y_sb = y_pool.tile([128, 2, N], BF16, tag="y")
                            y_map[jc2] = y_sb
                        y_sb = y_map[jc2]
                        off = qc * QC
                        for jcbit in range(2):
                            jc = 2 * jc2 + jcbit
                            psy = med_pool.tile([128, QC], F32, tag="med")
                            nc.tensor.matmul(
                                psy,
                                wo_sb[:, jc * 128 : (jc + 1) * 128],
                                saT[:, off : off + QC],
                                start=True, stop=True,
                            )
                            dst = y_sb[:, jcbit, off : off + QC]
                            # gpsimd cannot read PSUM; split PSUM drains
                            # between DVE (most) and ACT (a few)
                            if jcbit == 0 and jc2 < 2:
                                nc.scalar.activation(
                                    out=dst, in_=psy,
                                    func=mybir.ActivationFunctionType.Copy,
                                )
                            else:
                                nc.vector.tensor_copy(out=dst, in_=psy)
                        if qc == NQC - 1:
                            dma_eng = nc.sync if jc2 % 2 == 0 else nc.gpsimd
                            dma_eng.dma_start(
                                out=yt.ap()[:, 2 * jc2 : 2 * jc2 + 2, boff : boff + N],
                                in_=y_sb,
                            )
                    return run

                return [
                    [y_unit(qc, jc2) for jc2 in range(NJC2)] for qc in range(NQC)
                ]

            NSTEP = B * HLOC
            sa_map = {}
            pu0 = proj_units(0)
            # upfront: mk(0), mk(1), head-0 kq(jp=0), v(0, g=0)
            for idx0 in (0, 1, 2, 3, 10):
                pu0[idx0]()
            pu0_rest = pu0[4:10] + pu0[11:]
            pnext = None
            for i in range(NSTEP):
                b, l = divmod(i, HLOC)
                if l == 0:
                    saT_t = sa_pool.tile([HLOC * HD, N], BF16, tag="saT")
                    sa_map[b] = saT_t
                background = []
                if i == 0:
                    background += pu0_rest
                if i >= 1:
                    bprev, lprev = divmod(i - 1, HLOC)
                    if lprev == HLOC - 1:
                        for qunits in outproj_units(bprev):
                            background += qunits
                if l == 0 and b + 1 < B:
                    pnext = proj_units(b + 1)
                    background += pnext[:10]
                elif l == 1 and pnext is not None:
                    background += pnext[10:]
                if i == NSTEP - 1:
                    # last step: weave the final batch's output projection in
                    # as its q-chunks complete
                    oun = outproj_units(B - 1)

                    def on_qc(qc, _bg=background, _oun=oun):
                        _bg.extend(_oun[qc])

                    attn_emit(i, background, on_qc=on_qc)
                else:
                    attn_emit(i, background)

    nc.compile()
    return nc


_PROGRAM = None


def kernel(x, Wkqv, bkqv, Wout, bout):
    global LAST_RUN, _PROGRAM
    x = np.asarray(x, dtype=np.float32)
    Wkqv = np.asarray(Wkqv, dtype=np.float32)
    bkqv = np.asarray(bkqv, dtype=np.float32)
    Wout = np.asarray(Wout, dtype=np.float32)
    bout = np.asarray(bout, dtype=np.float32)

    scale = np.float32(1.0 / np.sqrt(HD))
    x2d = x.reshape(BN, D)

    in_maps = []
    for c in range(NCORES):
        h0 = c * HLOC
        # xt: [HLOC, 65, BN]; row 64 = ones (bias row for the projections)
        xt = np.empty((HLOC, HD + 1, BN), dtype=np.float32)
        for l in range(HLOC):
            xt[l, :HD] = x2d[:, (h0 + l) * HD : (h0 + l + 1) * HD].T
            xt[l, HD] = 1.0
        wk = np.empty((HD + 1, HLOC, HD), dtype=np.float32)
        wq = np.empty((HD + 1, HLOC, HD), dtype=np.float32)
        wv = np.zeros((HD + 1, HLOC, HD + 2), dtype=np.float32)
        for l in range(HLOC):
            h = h0 + l
            wk[:HD, l, :] = Wkqv[h][:, 0:HD]  # chunk order is (k, q, v)
            wk[HD, l, :] = bkqv[h][0:HD]
            wq[:HD, l, :] = Wkqv[h][:, HD : 2 * HD] * scale
            wq[HD, l, :] = bkqv[h][HD : 2 * HD] * scale
            wv[:HD, l, :HD] = Wkqv[h][:, 2 * HD : 3 * HD]
            wv[HD, l, :HD] = bkqv[h][2 * HD : 3 * HD]  # bias row
            wv[HD, l, HD] = 1.0  # ones column -> softmax denominator
        wo = np.ascontiguousarray(
            Wout[:, h0 * HD : (h0 + HLOC) * HD].T
        ).astype(BF16NP)

        rr, cc2 = np.arange(KT)[:, None], np.arange(KT)[None, :]
        msk = np.where(cc2 < rr, np.float32(-40.0), np.float32(0.0))

        in_maps.append(
            {
                "xt": xt,
                "xtb": xt.astype(BF16NP),
                "wk": wk,
                "wq": wq,
                "wv": wv.astype(BF16NP),
                "wo": wo,
                "one64": np.ones((1, HD), dtype=np.float32).astype(BF16NP),
                "msk": msk.astype(BF16NP),
                "i128": np.eye(KT, dtype=np.float32).astype(BF16NP),
            }
        )

    if _PROGRAM is None:
        _PROGRAM = _build_program()
    LAST_RUN = run_bass_kernel_spmd(_PROGRAM, in_maps, core_ids=list(range(NCORES)))

    y_t = np.zeros((D, BN), dtype=np.float32)
    for c in range(NCORES):
        ytc = LAST_RUN.results[c]["yt"].astype(np.float32)  # [128, 8, BN]
        y_t += ytc.transpose(1, 0, 2).reshape(D, BN)
    y = y_t.T + bout
    return y.reshape(B, N, D).astype(np.float32)


# revision 42
# speedup vs baseline: 1.1624x; 1.1624x over previous
"""Causal self-attention Trainium2 kernel (B=4, N=2048, D=1024, H=16, HD=64).

Sharding: tensor-parallel over heads — 8 cores x 2 heads each, all 4 batches.
Each core computes q/k/v projections for its 2 heads, causal-softmax
attention, and its partial contribution to the output projection
(sa_local @ Wout[:, cols].T). Host sums the 8 partials and adds bout.

Everything on-chip is kept "transposed" ([feature, token]) so no on-device
transposes are needed:
  - scores^T[k, q] = matmul(lhsT=kT_block, rhs=qT_chunk)
  - softmax denominator comes free as row 64 of the PV matmul by augmenting
    v with a ones column
  - U^T = v_aug^T @ expS^T accumulates over k-tiles in PSUM
  - out^T[j, n] = matmul(lhsT=WoutT_cols, rhs=saT)
k/q/v biases are folded into the projection matmuls via a ones row of x.
k, q, es, v, sa and the output-projection operands are bf16; the softmax
exp is skipped-max (scores ~N(0,1), softmax is shift-invariant).
"""

import os
import sys

for _p in ("/opt/trn_rl_repo", "/root/.axon_site/_ro/trn_rl_repo"):
    if os.path.isdir(_p) and _p not in sys.path:
        sys.path.insert(0, _p)
        break

import ml_dtypes
import numpy as np

import concourse.bacc as bacc
import concourse.tile as tile
from concourse import mybir
from concourse.bass_utils import run_bass_kernel_spmd

B, N, D, H = 4, 2048, 1024, 16
HD = D // H  # 64
NCORES = 8
HLOC = H // NCORES  # 2 local heads per core
BN = B * N  # 8192
QC = 512  # q-chunk width (PSUM bank)
KT = 128  # k-tile height
NQC = N // QC  # 4 q-chunks per batch
NKT = N // KT  # 16 k-tiles per batch

F32 = mybir.dt.float32
F32R = mybir.dt.float32r
BF16 = mybir.dt.bfloat16
BF16NP = ml_dtypes.bfloat16

LAST_RUN = None  # BassKernelResults of the most recent run (for test harness)


def _build_program():
    nc = bacc.Bacc("TRN2", num_devices=NCORES)

    # Per-core inputs (same shapes on every core, different values).
    xt = nc.dram_tensor("xt", [HLOC, HD + 1, BN], F32R, kind="ExternalInput")
    xtb = nc.dram_tensor("xtb", [HLOC, HD + 1, BN], BF16, kind="ExternalInput")
    wk = nc.dram_tensor("wk", [HD + 1, HLOC, HD], F32R, kind="ExternalInput")
    wq = nc.dram_tensor("wq", [HD + 1, HLOC, HD], F32R, kind="ExternalInput")
    wv = nc.dram_tensor("wv", [HD + 1, HLOC, HD + 2], BF16, kind="ExternalInput")
    wo = nc.dram_tensor("wo", [HLOC * HD, D], BF16, kind="ExternalInput")
    # -40 strictly above the causal diagonal of a 128x128 block (c < r),
    # 0 elsewhere; added to diagonal score blocks pre-exp via I128.T @ msk.
    msk = nc.dram_tensor("msk", [KT, KT], BF16, kind="ExternalInput")
    i128 = nc.dram_tensor("i128", [KT, KT], BF16, kind="ExternalInput")
    # yt layout: [partition, out-block jc, token] so one DMA can cover
    # multiple jc blocks with a partition-major access pattern.
    yt = nc.dram_tensor("yt", [128, D // 128, BN], BF16, kind="ExternalOutput")

    with tile.TileContext(nc) as tc:
        with (
            nc.allow_low_precision(reason="bf16/f32r attention pipeline"),
            tc.tile_pool(name="const", bufs=1) as const,
            tc.tile_pool(name="kq", bufs=2) as kq_pool,
            tc.tile_pool(name="vp", bufs=2) as v_pool,
            tc.tile_pool(name="xp", bufs=2) as x_pool,
            tc.tile_pool(name="es", bufs=9) as es_pool,
            tc.tile_pool(name="sa", bufs=2) as sa_pool,
            tc.tile_pool(name="rq", bufs=3) as rq_pool,
            tc.tile_pool(name="yout", bufs=4) as y_pool,
            tc.tile_pool(name="pbig", bufs=2, space="PSUM") as big_pool,
            tc.tile_pool(name="pmed", bufs=2, space="PSUM") as med_pool,
            tc.tile_pool(name="psu", bufs=2, space="PSUM") as psu_pool,
        ):
            # --- resident weight tiles (only wk/wq block the first step's
            # projections; the rest load via the gpsimd DGE queue) ---
            wk_sb = const.tile([HD + 1, HLOC, HD], F32R, tag="wk")
            nc.sync.dma_start(out=wk_sb, in_=wk.ap())
            wq_sb = const.tile([HD + 1, HLOC, HD], F32R, tag="wq")
            nc.scalar.dma_start(out=wq_sb, in_=wq.ap())
            msk_sb = const.tile([KT, KT], BF16, tag="msk")
            nc.gpsimd.dma_start(out=msk_sb, in_=msk.ap())
            i128_sb = const.tile([KT, KT], BF16, tag="i128")
            nc.gpsimd.dma_start(out=i128_sb, in_=i128.ap())
            wv_sb = const.tile([HD + 1, HLOC, HD + 2], BF16, tag="wv")
            nc.gpsimd.dma_start(out=wv_sb, in_=wv.ap())
            wo_sb = const.tile([HLOC * HD, D], BF16, tag="wo")
            nc.gpsimd.dma_start(out=wo_sb, in_=wo.ap())

            # Per-batch SBUF state (k/q hold both heads stacked on 128
            # partitions), created when proj units are emitted.
            stb = {}

            def proj_units(b):
                """k/q/v projections for batch b (both heads) as a list of
                closures (one PSUM slot each) to interleave with the
                previous batch's attention."""
                boff = b * N
                state = {"xl": {}, "xlb": {}, "v": {}}
                stb[b] = state

                def mk(l):
                    def run():
                        xl = x_pool.tile([HD + 1, N], F32R, tag=f"xt{l}")
                        h = N // 2
                        nc.sync.dma_start(
                            out=xl[:, 0:h], in_=xt.ap()[l][:, boff : boff + h]
                        )
                        nc.sync.dma_start(
                            out=xl[:, h:N],
                            in_=xt.ap()[l][:, boff + h : boff + N],
                        )
                        xlb = x_pool.tile([HD + 1, N], BF16, tag=f"xtb{l}")
                        nc.scalar.dma_start(
                            out=xlb, in_=xtb.ap()[l][:, boff : boff + N]
                        )
                        state["xl"][l] = xl
                        state["xlb"][l] = xlb
                        klt = kq_pool.tile([HD, N], BF16, tag=f"k{l}")
                        qlt = kq_pool.tile([HD, N], BF16, tag=f"q{l}")
                        state[f"k{l}"] = klt
                        state[f"q{l}"] = qlt
                        vlt = v_pool.tile([KT, NKT, HD + 1], BF16, tag=f"v{l}")
                        state["v"][l] = vlt
                    return run

                def kq_unit(jp, which, l):
                    def run():
                        dst = state[f"{which}{l}"]
                        w_sb = wk_sb if which == "k" else wq_sb
                        psk = big_pool.tile([HD, 2 * QC], F32, tag="big")
                        for half in range(2):
                            j = 2 * jp + half
                            sl = slice(j * QC, (j + 1) * QC)
                            osl = slice(half * QC, (half + 1) * QC)
                            nc.tensor.matmul(
                                psk[:, osl],
                                w_sb[:, l, :],
                                state["xl"][l][:, sl],
                                start=True, stop=True,
                            )
                        ksl = slice(2 * jp * QC, 2 * (jp + 1) * QC)
                        nc.vector.tensor_copy(out=dst[:, ksl], in_=psk)
                    return run

                def v_unit(l, g):
                    def run():
                        psv = med_pool.tile([KT, 4, KT], F32, tag="med")
                        for gg in range(4):
                            kj = 4 * g + gg
                            nc.tensor.matmul(
                                psv[:, gg, 0 : HD + 2],
                                state["xlb"][l][:, kj * KT : (kj + 1) * KT],
                                wv_sb[:, l, :],
                                start=True, stop=True,
                            )
                        nc.vector.tensor_copy(
                            out=state["v"][l][:, 4 * g : 4 * (g + 1), :],
                            in_=psv[:, :, 0 : HD + 1],
                        )
                    return run

                units = [mk(0), mk(1)]
                units += [
                    kq_unit(jp, w, l)
                    for l in range(HLOC)
                    for jp in range(NQC // 2)
                    for w in ("k", "q")
                ]
                units += [v_unit(l, g) for l in range(HLOC) for g in range(NKT // 4)]
                return units

            # diag block (j, t) -> slot index; slots 0-7 live in a big PSUM
            # tile, slots 8-9 in a med tile. t == j is the triangular block.
            TRI = (0, 1, 3, 6)

            def attn_emit(i, background, on_qc=None):
                """Attention for step i; pops background units between
                score/PV pairs."""
                b, l = divmod(i, HLOC)
                state = stb[b]
                k_sb = state[f"k{l}"]
                q_sb = state[f"q{l}"]
                v_sb = state["v"][l]

                def emit_scores(unit):
                    kind, qc, t2 = unit
                    qsl = slice(qc * QC, (qc + 1) * QC)
                    if kind == "full":
                        pss = big_pool.tile([KT, 2 * QC], F32, tag="big")
                        es = es_pool.tile([KT, 2 * QC], BF16, tag="es")
                        for half in range(2):
                            kj = 2 * t2 + half
                            nc.tensor.matmul(
                                pss[:, half * QC : (half + 1) * QC],
                                k_sb[:, kj * KT : (kj + 1) * KT],
                                q_sb[:, qsl],
                                start=True, stop=True,
                            )
                        nc.scalar.activation(
                            out=es, in_=pss, func=mybir.ActivationFunctionType.Exp
                        )
                        return es
                    # fine diagonal unit: 10 [128,128] blocks (j, t<=j) for
                    # q columns j of this q-chunk; triangular blocks get a
                    # -40 additive mask via a matmul before exp.
                    pssb = big_pool.tile([KT, 8, KT], F32, tag="big")
                    pssm = med_pool.tile([KT, 4, KT], F32, tag="med")
                    esb = es_pool.tile([KT, 8, KT], BF16, tag="es")
                    esm = es_pool.tile([KT, 2, KT], BF16, tag="esd")
                    for j in range(4):
                        q0 = qc * QC + j * KT
                        for t in range(j + 1):
                            s = TRI[j] + t
                            dst = pssb[:, s, :] if s < 8 else pssm[:, s - 8, :]
                            kj = 4 * qc + t
                            nc.tensor.matmul(
                                dst,
                                k_sb[:, kj * KT : (kj + 1) * KT],
                                q_sb[:, q0 : q0 + KT],
                                start=True, stop=(t != j),
                            )
                            if t == j:
                                nc.tensor.matmul(
                                    dst, i128_sb, msk_sb,
                                    start=False, stop=True,
                                )
                    nc.scalar.activation(
                        out=esb, in_=pssb,
                        func=mybir.ActivationFunctionType.Exp,
                    )
                    nc.scalar.activation(
                        out=esm, in_=pssm[:, 0:2, :],
                        func=mybir.ActivationFunctionType.Exp,
                    )
                    return (esb, esm)

                psu_map = {}

                def emit_pv(unit, es):
                    kind, qc, t2 = unit
                    qsl = slice(qc * QC, (qc + 1) * QC)
                    if kind == "full":
                        for half in range(2):
                            kj = 2 * t2 + half
                            nc.tensor.matmul(
                                psu_map[qc],
                                v_sb[:, kj, :],
                                es[:, half * QC : (half + 1) * QC],
                                start=(kj == 0),
                                stop=False,
                            )
                        return
                    esb, esm = es
                    for j in range(4):
                        for t in range(j + 1):
                            s = TRI[j] + t
                            src = esb[:, s, :] if s < 8 else esm[:, s - 8, :]
                            kj = 4 * qc + t
                            nc.tensor.matmul(
                                psu_map[qc][:, j * KT : (j + 1) * KT],
                                v_sb[:, kj, :],
                                src,
                                start=(kj == 0),
                                # for qc 0 each 128-col region is its own
                                # accumulation group (no preceding fulls) —
                                # close it at the column's last block
                                stop=(j == 3 and t == 3)
                                or (qc == 0 and t == j),
                            )
                    if True:
                        # softmax denominator is row 64 of psu; normalize the
                        # finished q-chunk straight out of PSUM into saT.
                        # engine ops misbehave at partition offsets on this
                        # backend, so: evacuate psu to SBUF at base 0, DMA
                        # the denominator row down to partition 0, invert it
                        # there, and broadcast via a ones-column matmul.
                        psu = psu_map[qc]
                        saT = sa_map[b]
                        u65 = rq_pool.tile([HD + 1, QC], F32, tag="u65")
                        nc.vector.tensor_copy(out=u65, in_=psu)
                        dn = rq_pool.tile([1, QC], F32, tag="dn")
                        nc.gpsimd.dma_start(out=dn, in_=u65[HD : HD + 1, :])
                        rq = rq_pool.tile([1, QC], F32, tag="rq")
                        nc.vector.reciprocal_approx_fast(out=rq, in_=dn)
                        # broadcast 1/den across 64 partitions on gpsimd
                        # (verified: partition_broadcast works from p0)
                        psb = rq_pool.tile([HD, QC], F32, tag="psbb")
                        nc.gpsimd.partition_broadcast(
                            out_ap=psb, in_ap=rq, channels=HD
                        )
                        if l == 0:
                            nc.vector.tensor_mul(
                                out=saT[0:HD, qsl], in0=u65[0:HD, :], in1=psb
                            )
                        else:
                            # DVE lanes cannot shift partitions: stage head 1
                            # in SBUF and DMA-shift into rows 64-127 of saT.
                            sa_tmp = rq_pool.tile([HD, QC], BF16, tag="sat")
                            nc.vector.tensor_mul(
                                out=sa_tmp, in0=u65[0:HD, :], in1=psb
                            )
                            nc.gpsimd.dma_start(
                                out=saT[HD : 2 * HD, qsl], in_=sa_tmp
                            )
                        if on_qc is not None:
                            on_qc(qc)

                work = []
                for qc in range(NQC):
                    work += [("full", qc, t2) for t2 in range(2 * qc)]
                    work += [("diag", qc, 0)]
                npairs = len(work)
                pending = []
                for idx, unit in enumerate(work):
                    qc = unit[1]
                    if qc not in psu_map:
                        psu_t = psu_pool.tile([HD + 1, QC], F32, tag="psu")
                        psu_map[qc] = psu_t
                    if len(pending) >= 7:
                        u0, e0 = pending.pop(0)
                        emit_pv(u0, e0)
                    es = emit_scores(unit)
                    pending.append((unit, es))
                    # interleave background work (other steps' proj/outproj)
                    remaining = npairs - idx
                    take = -(-len(background) // remaining)  # ceil
                    for _ in range(take):
                        if background:
                            background.pop(0)()
                for u0, e0 in pending:
                    emit_pv(u0, e0)
                while background:
                    background.pop(0)()

            NJC2 = D // 256  # 4 output jc-pairs

            def outproj_units(b):
                """Output projection for batch b: 16 units keyed (qc, jc2),
                released per q-chunk. Each jc2 owns a [128, 2, 2048] bf16
                tile, DMA'd after its qc=3 unit."""
                boff = b * N
                y_map = {}

                def y_unit(qc, jc2):
                    def run():
                        saT = sa_map[b]
                        if jc2 not in y_map:
                            y_sb = y_pool.tile([128, 2, N], BF16, tag="y")
                            y_map[jc2] = y_sb
                        y_sb = y_map[jc2]
                        off = qc * QC
                        for jcbit in range(2):
                            jc = 2 * jc2 + jcbit
                            psy = med_pool.tile([128, QC], F32, tag="med")
                            nc.tensor.matmul(
                                psy,
                                wo_sb[:, jc * 128 : (jc + 1) * 128],
                                saT[:, off : off + QC],
                                start=True, stop=True,
                            )
                            dst = y_sb[:, jcbit, off : off + QC]
                            # gpsimd cannot read PSUM; split PSUM drains
                            # between DVE (most) and ACT (a few)
                            if jcbit == 0 and jc2 < 2:
                                nc.scalar.activation(
                                    out=dst, in_=psy,
                                    func=mybir.ActivationFunctionType.Copy,
                                )
                            else:
                                nc.vector.tensor_copy(out=dst, in_=psy)
                        if qc == NQC - 1:
                            dma_eng = nc.sync if jc2 % 2 == 0 else nc.gpsimd
                            dma_eng.dma_start(
                                out=yt.ap()[:, 2 * jc2 : 2 * jc2 + 2, boff : boff + N],
                                in_=y_sb,
                            )
                    return run

                return [
                    [y_unit(qc, jc2) for jc2 in range(NJC2)] for qc in range(NQC)
                ]

            NSTEP = B * HLOC
            sa_map = {}
            pu0 = proj_units(0)
            # upfront: mk(0), mk(1), head-0 kq(jp=0), v(0, g=0)
            for idx0 in (0, 1, 2, 3, 10):
                pu0[idx0]()
            pu0_rest = pu0[4:10] + pu0[11:]
            pnext = None
            for i in range(NSTEP):
                b, l = divmod(i, HLOC)
                if l == 0:
                    saT_t = sa_pool.tile([HLOC * HD, N], BF16, tag="saT")
                    sa_map[b] = saT_t
                background = []
                if i == 0:
                    background += pu0_rest
                if i >= 1:
                    bprev, lprev = divmod(i - 1, HLOC)
                    if lprev == HLOC - 1:
                        for qunits in outproj_units(bprev):
                            background += qunits
                if l == 0 and b + 1 < B:
                    pnext = proj_units(b + 1)
                    background += pnext[:10]
                elif l == 1 and pnext is not None:
                    background += pnext[10:]
                if i == NSTEP - 1:
                    # last step: weave the final batch's output projection in
                    # as its q-chunks complete
                    oun = outproj_units(B - 1)

                    def on_qc(qc, _bg=background, _oun=oun):
                        _bg.extend(_oun[qc])

                    attn_emit(i, background, on_qc=on_qc)
                else:
                    attn_emit(i, background)

    nc.compile()
    return nc


_PROGRAM = None


def kernel(x, Wkqv, bkqv, Wout, bout):
    global LAST_RUN, _PROGRAM
    x = np.asarray(x, dtype=np.float32)
    Wkqv = np.asarray(Wkqv, dtype=np.float32)
    bkqv = np.asarray(bkqv, dtype=np.float32)
    Wout = np.asarray(Wout, dtype=np.float32)
    bout = np.asarray(bout, dtype=np.float32)

    scale = np.float32(1.0 / np.sqrt(HD))
    x2d = x.reshape(BN, D)

    in_maps = []
    for c in range(NCORES):
        h0 = c * HLOC
        # xt: [HLOC, 65, BN]; row 64 = ones (bias row for the projections)
        xt = np.empty((HLOC, HD + 1, BN), dtype=np.float32)
        for l in range(HLOC):
            xt[l, :HD] = x2d[:, (h0 + l) * HD : (h0 + l + 1) * HD].T
            xt[l, HD] = 1.0
        wk = np.empty((HD + 1, HLOC, HD), dtype=np.float32)
        wq = np.empty((HD + 1, HLOC, HD), dtype=np.float32)
        wv = np.zeros((HD + 1, HLOC, HD + 2), dtype=np.float32)
        for l in range(HLOC):
            h = h0 + l
            wk[:HD, l, :] = Wkqv[h][:, 0:HD]  # chunk order is (k, q, v)
            wk[HD, l, :] = bkqv[h][0:HD]
            wq[:HD, l, :] = Wkqv[h][:, HD : 2 * HD] * scale
            wq[HD, l, :] = bkqv[h][HD : 2 * HD] * scale
            wv[:HD, l, :HD] = Wkqv[h][:, 2 * HD : 3 * HD]
            wv[HD, l, :HD] = bkqv[h][2 * HD : 3 * HD]  # bias row
            wv[HD, l, HD] = 1.0  # ones column -> softmax denominator
        wo = np.ascontiguousarray(
            Wout[:, h0 * HD : (h0 + HLOC) * HD].T
        ).astype(BF16NP)

        rr, cc2 = np.arange(KT)[:, None], np.arange(KT)[None, :]
        msk = np.where(cc2 < rr, np.float32(-40.0), np.float32(0.0))

        in_maps.append(
            {
                "xt": xt,
                "xtb": xt.astype(BF16NP),
                "wk": wk,
                "wq": wq,
                "wv": wv.astype(BF16NP),
                "wo": wo,
                "msk": msk.astype(BF16NP),
                "i128": np.eye(KT, dtype=np.float32).astype(BF16NP),
            }
        )

    if _PROGRAM is None:
        _PROGRAM = _build_program()
    LAST_RUN = run_bass_kernel_spmd(_PROGRAM, in_maps, core_ids=list(range(NCORES)))

    y_t = np.zeros((D, BN), dtype=np.float32)
    for c in range(NCORES):
        ytc = LAST_RUN.results[c]["yt"].astype(np.float32)  # [128, 8, BN]
        y_t += ytc.transpose(1, 0, 2).reshape(D, BN)
    y = y_t.T + bout
    return y.reshape(B, N, D).astype(np.float32)


# revision 44
# speedup vs baseline: 1.1736x; 1.0097x over previous
"""Causal self-attention Trainium2 kernel (B=4, N=2048, D=1024, H=16, HD=64).

Sharding: tensor-parallel over heads — 8 cores x 2 heads each, all 4 batches.
Each core computes q/k/v projections for its 2 heads, causal-softmax
attention, and its partial contribution to the output projection
(sa_local @ Wout[:, cols].T). Host sums the 8 partials and adds bout.

Everything on-chip is kept "transposed" ([feature, token]) so no on-device
transposes are needed:
  - scores^T[k, q] = matmul(lhsT=kT_block, rhs=qT_chunk)
  - softmax denominator comes free as row 64 of the PV matmul by augmenting
    v with a ones column
  - U^T = v_aug^T @ expS^T accumulates over k-tiles in PSUM
  - out^T[j, n] = matmul(lhsT=WoutT_cols, rhs=saT)
k/q/v biases are folded into the projection matmuls via a ones row of x.
k, q, es, v, sa and the output-projection operands are bf16; the softmax
exp is skipped-max (scores ~N(0,1), softmax is shift-invariant).
"""

import os
import sys

for _p in ("/opt/trn_rl_repo", "/root/.axon_site/_ro/trn_rl_repo"):
    if os.path.isdir(_p) and _p not in sys.path:
        sys.path.insert(0, _p)
        break

import ml_dtypes
import numpy as np

import concourse.bacc as bacc
import concourse.tile as tile
from concourse import mybir
from concourse.bass_utils import run_bass_kernel_spmd

B, N, D, H = 4, 2048, 1024, 16
HD = D // H  # 64
NCORES = 8
HLOC = H // NCORES  # 2 local heads per core
BN = B * N  # 8192
QC = 512  # q-chunk width (PSUM bank)
KT = 128  # k-tile height
NQC = N // QC  # 4 q-chunks per batch
NKT = N // KT  # 16 k-tiles per batch

F32 = mybir.dt.float32
F32R = mybir.dt.float32r
BF16 = mybir.dt.bfloat16
BF16NP = ml_dtypes.bfloat16

LAST_RUN = None  # BassKernelResults of the most recent run (for test harness)


def _build_program():
    nc = bacc.Bacc("TRN2", num_devices=NCORES)

    # Per-core inputs (same shapes on every core, different values).
    xt = nc.dram_tensor("xt", [HLOC, HD + 1, BN], F32R, kind="ExternalInput")
    xtb = nc.dram_tensor("xtb", [HLOC, HD + 1, BN], BF16, kind="ExternalInput")
    wk = nc.dram_tensor("wk", [HD + 1, HLOC, HD], F32R, kind="ExternalInput")
    wq = nc.dram_tensor("wq", [HD + 1, HLOC, HD], F32R, kind="ExternalInput")
    wv = nc.dram_tensor("wv", [HD + 1, HLOC, HD + 2], BF16, kind="ExternalInput")
    wo = nc.dram_tensor("wo", [HLOC * HD, D], BF16, kind="ExternalInput")
    # -40 strictly above the causal diagonal of a 128x128 block (c < r),
    # 0 elsewhere; added to diagonal score blocks pre-exp via I128.T @ msk.
    msk = nc.dram_tensor("msk", [KT, KT], BF16, kind="ExternalInput")
    i128 = nc.dram_tensor("i128", [KT, KT], BF16, kind="ExternalInput")
    # yt layout: [partition, out-block jc, token] so one DMA can cover
    # multiple jc blocks with a partition-major access pattern.
    yt = nc.dram_tensor("yt", [128, D // 128, BN], BF16, kind="ExternalOutput")

    with tile.TileContext(nc) as tc:
        with (
            nc.allow_low_precision(reason="bf16/f32r attention pipeline"),
            tc.tile_pool(name="const", bufs=1) as const,
            tc.tile_pool(name="kq", bufs=2) as kq_pool,
            tc.tile_pool(name="vp", bufs=2) as v_pool,
            tc.tile_pool(name="xp", bufs=2) as x_pool,
            tc.tile_pool(name="es", bufs=9) as es_pool,
            tc.tile_pool(name="sa", bufs=2) as sa_pool,
            tc.tile_pool(name="rq", bufs=3) as rq_pool,
            tc.tile_pool(name="yout", bufs=4) as y_pool,
            tc.tile_pool(name="pbig", bufs=2, space="PSUM") as big_pool,
            tc.tile_pool(name="pmed", bufs=2, space="PSUM") as med_pool,
            tc.tile_pool(name="psu", bufs=2, space="PSUM") as psu_pool,
        ):
            # --- resident weight tiles (only wk/wq block the first step's
            # projections; the rest load via the gpsimd DGE queue) ---
            wk_sb = const.tile([HD + 1, HLOC, HD], F32R, tag="wk")
            nc.sync.dma_start(out=wk_sb, in_=wk.ap())
            wq_sb = const.tile([HD + 1, HLOC, HD], F32R, tag="wq")
            nc.scalar.dma_start(out=wq_sb, in_=wq.ap())
            msk_sb = const.tile([KT, KT], BF16, tag="msk")
            nc.gpsimd.dma_start(out=msk_sb, in_=msk.ap())
            i128_sb = const.tile([KT, KT], BF16, tag="i128")
            nc.gpsimd.dma_start(out=i128_sb, in_=i128.ap())
            wv_sb = const.tile([HD + 1, HLOC, HD + 2], BF16, tag="wv")
            nc.gpsimd.dma_start(out=wv_sb, in_=wv.ap())
            wo_sb = const.tile([HLOC * HD, D], BF16, tag="wo")
            nc.gpsimd.dma_start(out=wo_sb, in_=wo.ap())

            # Per-batch SBUF state (k/q hold both heads stacked on 128
            # partitions), created when proj units are emitted.
            stb = {}

            def proj_units(b):
                """k/q/v projections for batch b (both heads) as a list of
                closures (one PSUM slot each) to interleave with the
                previous batch's attention."""
                boff = b * N
                state = {"xl": {}, "xlb": {}, "v": {}}
                stb[b] = state

                def mk(l):
                    def run():
                        xl = x_pool.tile([HD + 1, N], F32R, tag=f"xt{l}")
                        h = N // 2
                        nc.sync.dma_start(
                            out=xl[:, 0:h], in_=xt.ap()[l][:, boff : boff + h]
                        )
                        nc.sync.dma_start(
                            out=xl[:, h:N],
                            in_=xt.ap()[l][:, boff + h : boff + N],
                        )
                        xlb = x_pool.tile([HD + 1, N], BF16, tag=f"xtb{l}")
                        nc.scalar.dma_start(
                            out=xlb, in_=xtb.ap()[l][:, boff : boff + N]
                        )
                        state["xl"][l] = xl
                        state["xlb"][l] = xlb
                        klt = kq_pool.tile([HD, N], BF16, tag=f"k{l}")
                        qlt = kq_pool.tile([HD, N], BF16, tag=f"q{l}")
                        state[f"k{l}"] = klt
                        state[f"q{l}"] = qlt
                        vlt = v_pool.tile([KT, NKT, HD + 1], BF16, tag=f"v{l}")
                        state["v"][l] = vlt
                    return run

                def kq_unit(jp, which, l):
                    def run():
                        dst = state[f"{which}{l}"]
                        w_sb = wk_sb if which == "k" else wq_sb
                        psk = big_pool.tile([HD, 2 * QC], F32, tag="big")
                        for half in range(2):
                            j = 2 * jp + half
                            sl = slice(j * QC, (j + 1) * QC)
                            osl = slice(half * QC, (half + 1) * QC)
                            nc.tensor.matmul(
                                psk[:, osl],
                                w_sb[:, l, :],
                                state["xl"][l][:, sl],
                                start=True, stop=True,
                            )
                        ksl = slice(2 * jp * QC, 2 * (jp + 1) * QC)
                        nc.vector.tensor_copy(out=dst[:, ksl], in_=psk)
                    return run

                def v_unit(l, g):
                    def run():
                        psv = med_pool.tile([KT, 4, KT], F32, tag="med")
                        for gg in range(4):
                            kj = 4 * g + gg
                            nc.tensor.matmul(
                                psv[:, gg, 0 : HD + 2],
                                state["xlb"][l][:, kj * KT : (kj + 1) * KT],
                                wv_sb[:, l, :],
                                start=True, stop=True,
                            )
                        nc.vector.tensor_copy(
                            out=state["v"][l][:, 4 * g : 4 * (g + 1), :],
                            in_=psv[:, :, 0 : HD + 1],
                        )
                    return run

                units = [mk(0), mk(1)]
                units += [
                    kq_unit(jp, w, l)
                    for l in range(HLOC)
                    for jp in range(NQC // 2)
                    for w in ("k", "q")
                ]
                units += [v_unit(l, g) for l in range(HLOC) for g in range(NKT // 4)]
                return units

            # diag block (j, t) -> slot index; slots 0-7 live in a big PSUM
            # tile, slots 8-9 in a med tile. t == j is the triangular block.
            TRI = (0, 1, 3, 6)

            def attn_emit(i, background, on_qc=None, qc_order=None):
                """Attention for step i; pops background units between
                score/PV pairs."""
                b, l = divmod(i, HLOC)
                state = stb[b]
                k_sb = state[f"k{l}"]
                q_sb = state[f"q{l}"]
                v_sb = state["v"][l]

                def emit_scores(unit):
                    kind, qc, t2 = unit
                    qsl = slice(qc * QC, (qc + 1) * QC)
                    if kind == "full":
                        pss = big_pool.tile([KT, 2 * QC], F32, tag="big")
                        es = es_pool.tile([KT, 2 * QC], BF16, tag="es")
                        for half in range(2):
                            kj = 2 * t2 + half
                            nc.tensor.matmul(
                                pss[:, half * QC : (half + 1) * QC],
                                k_sb[:, kj * KT : (kj + 1) * KT],
                                q_sb[:, qsl],
                                start=True, stop=True,
                            )
                        nc.scalar.activation(
                            out=es, in_=pss, func=mybir.ActivationFunctionType.Exp
                        )
                        return es
                    # fine diagonal unit: 10 [128,128] blocks (j, t<=j) for
                    # q columns j of this q-chunk; triangular blocks get a
                    # -40 additive mask via a matmul before exp.
                    pssb = big_pool.tile([KT, 8, KT], F32, tag="big")
                    pssm = med_pool.tile([KT, 4, KT], F32, tag="med")
                    esb = es_pool.tile([KT, 8, KT], BF16, tag="es")
                    esm = es_pool.tile([KT, 2, KT], BF16, tag="esd")
                    for j in range(4):
                        q0 = qc * QC + j * KT
                        for t in range(j + 1):
                            s = TRI[j] + t
                            dst = pssb[:, s, :] if s < 8 else pssm[:, s - 8, :]
                            kj = 4 * qc + t
                            nc.tensor.matmul(
                                dst,
                                k_sb[:, kj * KT : (kj + 1) * KT],
                                q_sb[:, q0 : q0 + KT],
                                start=True, stop=(t != j),
                            )
                            if t == j:
                                nc.tensor.matmul(
                                    dst, i128_sb, msk_sb,
                                    start=False, stop=True,
                                )
                    nc.scalar.activation(
                        out=esb, in_=pssb,
                        func=mybir.ActivationFunctionType.Exp,
                    )
                    nc.scalar.activation(
                        out=esm, in_=pssm[:, 0:2, :],
                        func=mybir.ActivationFunctionType.Exp,
                    )
                    return (esb, esm)

                psu_map = {}

                def emit_pv(unit, es):
                    kind, qc, t2 = unit
                    qsl = slice(qc * QC, (qc + 1) * QC)
                    if kind == "full":
                        for half in range(2):
                            kj = 2 * t2 + half
                            nc.tensor.matmul(
                                psu_map[qc],
                                v_sb[:, kj, :],
                                es[:, half * QC : (half + 1) * QC],
                                start=(kj == 0),
                                stop=False,
                            )
                        return
                    esb, esm = es
                    for j in range(4):
                        for t in range(j + 1):
                            s = TRI[j] + t
                            src = esb[:, s, :] if s < 8 else esm[:, s - 8, :]
                            kj = 4 * qc + t
                            nc.tensor.matmul(
                                psu_map[qc][:, j * KT : (j + 1) * KT],
                                v_sb[:, kj, :],
                                src,
                                start=(kj == 0),
                                # for qc 0 each 128-col region is its own
                                # accumulation group (no preceding fulls) —
                                # close it at the column's last block
                                stop=(j == 3 and t == 3)
                                or (qc == 0 and t == j),
                            )
                    if True:
                        # softmax denominator is row 64 of psu; normalize the
                        # finished q-chunk straight out of PSUM into saT.
                        # engine ops misbehave at partition offsets on this
                        # backend, so: evacuate psu to SBUF at base 0, DMA
                        # the denominator row down to partition 0, invert it
                        # there, and broadcast via a ones-column matmul.
                        psu = psu_map[qc]
                        saT = sa_map[b]
                        u65 = rq_pool.tile([HD + 1, QC], F32, tag="u65")
                        nc.vector.tensor_copy(out=u65, in_=psu)
                        dn = rq_pool.tile([1, QC], F32, tag="dn")
                        nc.gpsimd.dma_start(out=dn, in_=u65[HD : HD + 1, :])
                        rq = rq_pool.tile([1, QC], F32, tag="rq")
                        nc.vector.reciprocal_approx_fast(out=rq, in_=dn)
                        # broadcast 1/den across 64 partitions on gpsimd
                        # (verified: partition_broadcast works from p0)
                        psb = rq_pool.tile([HD, QC], F32, tag="psbb")
                        nc.gpsimd.partition_broadcast(
                            out_ap=psb, in_ap=rq, channels=HD
                        )
                        if l == 0:
                            nc.vector.tensor_mul(
                                out=saT[0:HD, qsl], in0=u65[0:HD, :], in1=psb
                            )
                        else:
                            # DVE lanes cannot shift partitions: stage head 1
                            # in SBUF and DMA-shift into rows 64-127 of saT.
                            sa_tmp = rq_pool.tile([HD, QC], BF16, tag="sat")
                            nc.vector.tensor_mul(
                                out=sa_tmp, in0=u65[0:HD, :], in1=psb
                            )
                            nc.gpsimd.dma_start(
                                out=saT[HD : 2 * HD, qsl], in_=sa_tmp
                            )
                        if on_qc is not None:
                            on_qc(qc)

                work = []
                for qc in qc_order if qc_order else range(NQC):
                    work += [("full", qc, t2) for t2 in range(2 * qc)]
                    work += [("diag", qc, 0)]
                npairs = len(work)
                pending = []
                for idx, unit in enumerate(work):
                    qc = unit[1]
                    if qc not in psu_map:
                        psu_t = psu_pool.tile([HD + 1, QC], F32, tag="psu")
                        psu_map[qc] = psu_t
                    if len(pending) >= 7:
                        u0, e0 = pending.pop(0)
                        emit_pv(u0, e0)
                    es = emit_scores(unit)
                    pending.append((unit, es))
                    # interleave background work (other steps' proj/outproj)
                    remaining = npairs - idx
                    take = -(-len(background) // remaining)  # ceil
                    for _ in range(take):
                        if background:
                            background.pop(0)()
                for u0, e0 in pending:
                    emit_pv(u0, e0)
                while background:
                    background.pop(0)()

            NJC2 = D // 256  # 4 output jc-pairs

            def outproj_units(b, last=False):
                """Output projection for batch b: 16 units keyed (qc, jc2),
                released per q-chunk (any order). Each jc2 owns a
                [128, 2, 2048] bf16 tile, DMA'd once its 4 chunks are done."""
                boff = b * N
                y_map = {}
                done = {jc2: 0 for jc2 in range(NJC2)}

                def y_unit(qc, jc2):
                    def run():
                        saT = sa_map[b]
                        if jc2 not in y_map:
                            y_sb = y_pool.tile([128, 2, N], BF16, tag="y")
                            y_map[jc2] = y_sb
                        y_sb = y_map[jc2]
                        off = qc * QC
                        for jcbit in range(2):
                            jc = 2 * jc2 + jcbit
                            psy = med_pool.tile([128, QC], F32, tag="med")
                            nc.tensor.matmul(
                                psy,
                                wo_sb[:, jc * 128 : (jc + 1) * 128],
                                saT[:, off : off + QC],
                                start=True, stop=True,
                            )
                            dst = y_sb[:, jcbit, off : off + QC]
                            # gpsimd cannot read PSUM; split PSUM drains
                            # between DVE (most) and ACT (more when the
                            # final batch winds down and ACT sits idle)
                            on_act = (
                                jcbit == 0 if last else (jcbit == 0 and jc2 < 2)
                            )
                            if on_act:
                                nc.scalar.activation(
                                    out=dst, in_=psy,
                                    func=mybir.ActivationFunctionType.Copy,
                                )
                            else:
                                nc.vector.tensor_copy(out=dst, in_=psy)
                        done[jc2] += 1
                        if done[jc2] == NQC:
                            dma_eng = nc.sync if jc2 % 2 == 0 else nc.gpsimd
                            dma_eng.dma_start(
                                out=yt.ap()[:, 2 * jc2 : 2 * jc2 + 2, boff : boff + N],
                                in_=y_sb,
                            )
                    return run

                return [
                    [y_unit(qc, jc2) for jc2 in range(NJC2)] for qc in range(NQC)
                ]

            NSTEP = B * HLOC
            sa_map = {}
            pu0 = proj_units(0)
            # upfront: mk(0), mk(1), head-0 kq(jp=0), v(0, g=0)
            for idx0 in (0, 1, 2, 3, 10):
                pu0[idx0]()
            pu0_rest = pu0[4:10] + pu0[11:]
            pnext = None
            for i in range(NSTEP):
                b, l = divmod(i, HLOC)
                if l == 0:
                    saT_t = sa_pool.tile([HLOC * HD, N], BF16, tag="saT")
                    sa_map[b] = saT_t
                background = []
                if i == 0:
                    background += pu0_rest
                if i >= 1:
                    bprev, lprev = divmod(i - 1, HLOC)
                    if lprev == HLOC - 1:
                        for qunits in outproj_units(bprev):
                            background += qunits
                if l == 0 and b + 1 < B:
                    pnext = proj_units(b + 1)
                    background += pnext[:10]
                elif l == 1 and pnext is not None:
                    background += pnext[10:]
                if i == NSTEP - 1:
                    # last step: process the big q-chunks first and weave the
                    # final batch's output projection in as chunks complete,
                    # so the tail ends on the smallest chunk's chain
                    oun = outproj_units(B - 1, last=True)

                    def on_qc(qc, _bg=background, _oun=oun):
                        _bg.extend(_oun[qc])

                    attn_emit(i, background, on_qc=on_qc)
                else:
                    attn_emit(i, background)

    nc.compile()
    return nc


_PROGRAM = None


def kernel(x, Wkqv, bkqv, Wout, bout):
    global LAST_RUN, _PROGRAM
    x = np.asarray(x, dtype=np.float32)
    Wkqv = np.asarray(Wkqv, dtype=np.float32)
    bkqv = np.asarray(bkqv, dtype=np.float32)
    Wout = np.asarray(Wout, dtype=np.float32)
    bout = np.asarray(bout, dtype=np.float32)

    scale = np.float32(1.0 / np.sqrt(HD))
    x2d = x.reshape(BN, D)

    in_maps = []
    for c in range(NCORES):
        h0 = c * HLOC
        # xt: [HLOC, 65, BN]; row 64 = ones (bias row for the projections)
        xt = np.empty((HLOC, HD + 1, BN), dtype=np.float32)
        for l in range(HLOC):
            xt[l, :HD] = x2d[:, (h0 + l) * HD : (h0 + l + 1) * HD].T
            xt[l, HD] = 1.0
        wk = np.empty((HD + 1, HLOC, HD), dtype=np.float32)
        wq = np.empty((HD + 1, HLOC, HD), dtype=np.float32)
        wv = np.zeros((HD + 1, HLOC, HD + 2), dtype=np.float32)
        for l in range(HLOC):
            h = h0 + l
            wk[:HD, l, :] = Wkqv[h][:, 0:HD]  # chunk order is (k, q, v)
            wk[HD, l, :] = bkqv[h][0:HD]
            wq[:HD, l, :] = Wkqv[h][:, HD : 2 * HD] * scale
            wq[HD, l, :] = bkqv[h][HD : 2 * HD] * scale
            wv[:HD, l, :HD] = Wkqv[h][:, 2 * HD : 3 * HD]
            wv[HD, l, :HD] = bkqv[h][2 * HD : 3 * HD]  # bias row
            wv[HD, l, HD] = 1.0  # ones column -> softmax denominator
        wo = np.ascontiguousarray(
            Wout[:, h0 * HD : (h0 + HLOC) * HD].T
        ).astype(BF16NP)

        rr, cc2 = np.arange(KT)[:, None], np.arange(KT)[None, :]
        msk = np.where(cc2 < rr, np.float32(-40.0), np.float32(0.0))

        in_maps.append(
            {
                "xt": xt,
                "xtb": xt.astype(BF16NP),
                "wk": wk,
                "wq": wq,
                "wv": wv.astype(BF16NP),
                "wo": wo,
                "msk": msk.astype(BF16NP),
                "i128": np.eye(KT, dtype=np.float32).astype(BF16NP),
            }
        )

    if _PROGRAM is None:
        _PROGRAM = _build_program()
    LAST_RUN = run_bass_kernel_spmd(_PROGRAM, in_maps, core_ids=list(range(NCORES)))

    y_t = np.zeros((D, BN), dtype=np.float32)
    for c in range(NCORES):
        ytc = LAST_RUN.results[c]["yt"].astype(np.float32)  # [128, 8, BN]
        y_t += ytc.transpose(1, 0, 2).reshape(D, BN)
    y = y_t.T + bout
    return y.reshape(B, N, D).astype(np.float32)
